# revision 1
# baseline (speedup 1.0000x reference)
"""Trainium2 Bass kernel for nn_MultiHeadCDGCN.

Math (per batch b):
  t_w  = softmax(x, axis=T);  TAtt = sum_T(x * t_w)          [N, D]
  Q    = x @ W_Q.T                                           [T, N, D]
  K    = TAtt @ W_K.T ; V = TAtt @ W_V.T                     [N, D]
  S_th = Q_th @ K_h.T / sqrt(dh)   (per t, head h)           [N, N]
  out  = (relu(S) + I) @ V = relu(S) @ V + V                 [T, N, D]

Sharding: data-parallel over B across 8 NeuronCores (B == 8, one batch
per core); no collectives.

Notes on structure:
  - Built on Bacc (not plain Bass) so excess per-instruction semaphore
    waits are legalized onto EventSemaphore/Ldweights instructions
    (TRN2 allows 1 wait per instruction).
  - S is computed into 2-bank [128, 1024] PSUM tiles (two heads per
    tile) so relu evacuation amortizes the per-instruction overhead.
  - A@V accumulates all four column tiles concurrently into disjoint
    partition quadrants of one PSUM bank (skip_group_check: the
    conservative whole-bank group check would serialize them; HW
    has_written is per-element).
  - All matmuls are fp32 (f32r was measured 4x faster on S but its
    ~1e-4 relative error is ~100x the fp32 envelope; kept exact).
  - Attention matmuls use PE array tiling: S with 32x128 row tiles
    (K = dh = 32), A@V with 128x32 column tiles (M = dh = 32), 4 heads
    resident concurrently.
"""

import sys

import numpy as np

sys.path.insert(0, "/opt/trn_rl_repo")

import concourse.bacc as bacc  # noqa: E402
import concourse.tile as tile  # noqa: E402
from concourse import mybir  # noqa: E402
from concourse.masks import make_identity  # noqa: E402
from concourse.bass_utils import run_bass_kernel_spmd  # noqa: E402

F32 = mybir.dt.float32
F32R = mybir.dt.float32r
AF = mybir.ActivationFunctionType

B, T, N, D, H, DH = 8, 32, 256, 256, 8, 32
P = 128
NCHUNKS = 16  # tn chunks of 512 (2 frames each)
CHUNK_T = 2  # frames per chunk
CHUNK_TN = CHUNK_T * N  # 512

_CACHE: dict = {}


def _build_program():
    nc = bacc.Bacc()

    x_d = nc.dram_tensor("x", [T, N, D], F32, kind="ExternalInput")
    wqt_d = nc.dram_tensor("wqt", [D, D], F32, kind="ExternalInput")
    wkt_d = nc.dram_tensor("wkt", [D, D], F32, kind="ExternalInput")
    wvt_d = nc.dram_tensor("wvt", [D, D], F32, kind="ExternalInput")
    out_d = nc.dram_tensor("out", [T, N, D], F32, kind="ExternalOutput")

    with tile.TileContext(nc) as tc:
        with (
            tc.tile_pool(name="consts", bufs=1) as consts,
            tc.tile_pool(name="xa", bufs=4) as xa_pool,
            tc.tile_pool(name="xt", bufs=3) as xt_pool,
            tc.tile_pool(name="ew", bufs=6) as e_pool,
            tc.tile_pool(name="at", bufs=10) as a_pool,
            tc.tile_pool(name="ot", bufs=6) as o_pool,
            tc.tile_pool(name="misc", bufs=2) as misc,
            tc.tile_pool(name="ps_a", bufs=3, space="PSUM") as ps_a,
            tc.tile_pool(name="ps_o", bufs=2, space="PSUM") as ps_o,
        ):
            eye = consts.tile([P, P], F32)
            make_identity(nc, eye)

            # Weights, [k, j] with k split over 2 partition tiles.
            wqt_sb = consts.tile([P, 2, D], F32)
            wkt_sb = consts.tile([P, 2, D], F32)
            wvt_sb = consts.tile([P, 2, D], F32)
            for w_sb, w_d in ((wqt_sb, wqt_d), (wkt_sb, wkt_d), (wvt_sb, wvt_d)):
                for kc in range(2):
                    nc.sync.dma_start(
                        out=w_sb[:, kc, :],
                        in_=w_d[kc * P : (kc + 1) * P, :].bitcast(w_sb.dtype),
                    )

            # Softmax-pool statistics in transposed [d, n] layout.
            sum_e = consts.tile([P, 2, N], F32)
            sum_xe = consts.tile([P, 2, N], F32)
            nc.gpsimd.memset(sum_e, 0.0)
            nc.gpsimd.memset(sum_xe, 0.0)

            # Q.T strip [j, tn] resident (j split over 2 partition tiles).
            qt_sb = consts.tile([P, 2, T * N], F32)

            # ---------------- Phase A: stream x, build x.T, stats, Q.T
            for c in range(NCHUNKS):
                t0 = c * CHUNK_T
                xa = xa_pool.tile([P, 4, D], F32)
                nc.sync.dma_start(
                    out=xa,
                    in_=x_d[t0 : t0 + CHUNK_T].rearrange(
                        "t (s p) d -> p (t s) d", p=P
                    ),
                )

                xt = xt_pool.tile([P, 2, CHUNK_TN], F32)
                for dc in range(2):
                    pt = ps_a.tile([P, CHUNK_TN], F32, tag="psa", name=f"pt{dc}")
                    for s in range(4):
                        nc.tensor.transpose(
                            pt[:, s * P : (s + 1) * P],
                            xa[:, s, dc * P : (dc + 1) * P],
                            eye,
                        )
                    nc.scalar.activation(xt[:, dc, :], pt, AF.Copy)
                    # Stats straight off the PSUM x.T chunk.
                    e_t = e_pool.tile([P, CHUNK_TN], F32)
                    nc.scalar.activation(e_t, pt, AF.Exp)
                    xe_t = e_pool.tile([P, CHUNK_TN], F32)
                    nc.vector.tensor_mul(xe_t, pt, e_t)
                    for ti in range(CHUNK_T):
                        nc.gpsimd.tensor_add(
                            sum_e[:, dc, :],
                            sum_e[:, dc, :],
                            e_t[:, ti * N : (ti + 1) * N],
                        )
                        nc.vector.tensor_add(
                            sum_xe[:, dc, :],
                            sum_xe[:, dc, :],
                            xe_t[:, ti * N : (ti + 1) * N],
                        )

                # Q.T chunk: [j, tn] = sum_k W_Q.T[k, j]^T x.T[k, tn]
                for jc in range(2):
                    pq = ps_a.tile([P, CHUNK_TN], F32, tag="psa", name=f"pq{jc}")
                    for kc in range(2):
                        nc.tensor.matmul(
                            pq,
                            wqt_sb[:, kc, jc * P : (jc + 1) * P],
                            xt[:, kc, :],
                            start=(kc == 0),
                            stop=(kc == 1),
                        )
                    if jc == 0:
                        nc.scalar.activation(
                            qt_sb[:, jc, c * CHUNK_TN : (c + 1) * CHUNK_TN],
                            pq,
                            AF.Copy,
                        )
                    else:
                        nc.vector.tensor_copy(
                            qt_sb[:, jc, c * CHUNK_TN : (c + 1) * CHUNK_TN], pq
                        )

            # ---------------- Phase B: TAtt.T, K.T, V, V.T
            rec = misc.tile([P, 2, N], F32)
            tatt_t = consts.tile([P, 2, N], F32)  # TAtt.T [d, n]
            for dc in range(2):
                nc.vector.reciprocal(rec[:, dc, :], sum_e[:, dc, :])
                nc.vector.tensor_mul(
                    tatt_t[:, dc, :], sum_xe[:, dc, :], rec[:, dc, :]
                )

            kt_sb = consts.tile([P, 2, N], F32)  # K.T [j, m] (pre-scaled)
            for jc in range(2):
                pk = ps_a.tile([P, N], F32, tag="psa", name="pk")
                for kc in range(2):
                    nc.tensor.matmul(
                        pk,
                        wkt_sb[:, kc, jc * P : (jc + 1) * P],
                        tatt_t[:, kc, :],
                        start=(kc == 0),
                        stop=(kc == 1),
                    )
                nc.vector.tensor_copy(kt_sb[:, jc, :], pk)

            v_sb = consts.tile([P, 2, D], F32)  # V [m, j]
            for mc in range(2):
                pv = ps_a.tile([P, D], F32, tag="psa", name="pv")
                for kc in range(2):
                    nc.tensor.matmul(
                        pv,
                        tatt_t[:, kc, mc * P : (mc + 1) * P],
                        wvt_sb[:, kc, :],
                        start=(kc == 0),
                        stop=(kc == 1),
                    )
                nc.vector.tensor_copy(v_sb[:, mc, :], pv)

            vt_sb = consts.tile([P, 2, N], F32)  # V.T [j, m]
            for jc in range(2):
                pt2 = ps_a.tile([P, N], F32, tag="psa", name="pt2")
                for mc in range(2):
                    nc.tensor.transpose(
                        pt2[:, mc * P : (mc + 1) * P],
                        v_sb[:, mc, jc * P : (jc + 1) * P],
                        eye,
                    )
                nc.vector.tensor_copy(vt_sb[:, jc, :], pt2)

            # ---------------- Phase C: attention + output
            # Both head-groups' S matmuls run as one row-tile burst, then
            # both A@V bursts (col tiles), halving PE array mode switches.
            for c in range(NCHUNKS):
                t0 = c * CHUNK_T
                a_str = {}
                nrelu = 0
                for hg in range(2):
                    for mc in range(2):
                        for rp in range(2):  # head pairs share a 2-bank tile
                            ps2 = ps_a.tile(
                                [P, 2 * CHUNK_TN],
                                F32,
                                tag="psa",
                                name=f"ps{hg}{mc}{rp}",
                            )
                            for rh in range(2):
                                r = rp * 2 + rh
                                nc.tensor.matmul(
                                    ps2[:, rh * CHUNK_TN : (rh + 1) * CHUNK_TN],
                                    kt_sb[
                                        r * 32 : (r + 1) * 32,
                                        hg,
                                        mc * P : (mc + 1) * P,
                                    ],
                                    qt_sb[
                                        r * 32 : (r + 1) * 32,
                                        hg,
                                        c * CHUNK_TN : (c + 1) * CHUNK_TN,
                                    ],
                                    start=True,
                                    stop=True,
                                    tile_position=(r * 32, 0),
                                )
                            a2 = a_pool.tile(
                                [P, 2 * CHUNK_TN],
                                F32,
                                tag="at",
                                name=f"a{hg}{mc}{rp}",
                            )
                            # Split relu evacuation ACT/DVE ~5:3.
                            if (c + nrelu) % 8 in (0, 3, 6):
                                nc.vector.tensor_scalar_max(a2, ps2, 0.0)
                            else:
                                nc.scalar.activation(a2, ps2, AF.Relu)
                            nrelu += 1
                            for rh in range(2):
                                a_str[(hg, rp * 2 + rh, mc)] = a2[
                                    :, rh * CHUNK_TN : (rh + 1) * CHUNK_TN
                                ]
                for hg in range(2):
                    po = ps_o.tile([P, CHUNK_TN], F32, tag="po", name=f"po{hg}")
                    # All four column tiles accumulate concurrently into
                    # disjoint partition quadrants of one PSUM bank.
                    for mc in range(2):
                        for r in range(4):
                            h = hg * 4 + r
                            nc.tensor.matmul(
                                po[r * 32 : (r + 1) * 32, :],
                                v_sb[:, mc, h * 32 : (h + 1) * 32],
                                a_str[(hg, r, mc)],
                                start=(mc == 0),
                                stop=(mc == 1),
                                tile_position=(0, r * 32),
                                skip_group_check=True,
                            )
                    o_sb = o_pool.tile([P, CHUNK_T, N], F32)
                    for ti in range(CHUNK_T):
                        nc.vector.scalar_tensor_tensor(
                            out=o_sb[:, ti, :],
                            in0=po[:, ti * N : (ti + 1) * N],
                            scalar=1.0,
                            in1=vt_sb[:, hg, :],
                            op0=mybir.AluOpType.mult,
                            op1=mybir.AluOpType.add,
                        )
                    o_str = o_pool.tile([P, CHUNK_T, N], F32)
                    nc.vector.transpose(o_str, o_sb)
                    for ti in range(CHUNK_T):
                        for r in range(4):
                            dma_eng = nc.sync if (ti * 4 + r) % 2 == 0 else nc.gpsimd
                            dma_eng.dma_start(
                                out=out_d[t0 + ti].rearrange(
                                    "(nb nn) (g r hd) -> g r nn nb hd",
                                    nn=32,
                                    g=2,
                                    hd=32,
                                )[hg, r],
                                in_=o_str[r * 32 : (r + 1) * 32, ti, :].rearrange(
                                    "p (nb hd) -> p nb hd", hd=32
                                ),
                            )

    nc.finalize()
    return nc


def kernel(**inputs) -> np.ndarray:
    x = np.ascontiguousarray(np.asarray(inputs["x"], dtype=np.float32))
    w_q = np.asarray(inputs["W_Q"], dtype=np.float32)
    w_k = np.asarray(inputs["W_K"], dtype=np.float32)
    w_v = np.asarray(inputs["W_V"], dtype=np.float32)

    if "nc" not in _CACHE:
        _CACHE["nc"] = _build_program()
    nc = _CACHE["nc"]

    wqt = np.ascontiguousarray(w_q.T)
    wkt = np.ascontiguousarray(w_k.T) * np.float32(1.0 / np.sqrt(DH))
    wvt = np.ascontiguousarray(w_v.T)

    in_maps = [
        {"x": np.ascontiguousarray(x[b]), "wqt": wqt, "wkt": wkt, "wvt": wvt}
        for b in range(B)
    ]
    res = run_bass_kernel_spmd(nc, in_maps, core_ids=list(range(B)))
    out = np.stack([res.results[b]["out"] for b in range(B)], axis=0)
    return out.reshape(B, T, N, D)

